# revision 33
# baseline (speedup 1.0000x reference)
# Multi-head attention (b=4, s=1024, d=1024, h=16, hd=64, no mask) on 8
# Trainium2 NeuronCores via Bass/Tile.
#
# Sharding: core c = (bi, g) with bi = c // 2 (batch), g = c % 2 (head group
# of 8 heads = 512 of the 1024 projection output dims).  Each core computes
# a partial out-projection for its batch; the host sums the two partials per
# batch and adds the bias.
#
# Per-core dataflow (bf16 operands, fp32 PSUM accumulation, fp32 output):
#   qT = WqT.T @ xT   -> [dj_local 512, si 1024]   (dj on partitions)
#   kT = WkT.T @ xT   -> same layout
#   v  = xT.T  @ WvT  -> [si 1024, dj_local 512]   (si on partitions)
#   scores per head pair: even head on PE rows 0-63, odd on rows 64-127 --
#     emitted adjacently so the two K=64 matmuls run on disjoint row-group
#     sub-arrays concurrently (full PE utilization despite hd=64)
#   exp via ScalarE (scale=1/8) from PSUM -> bf16 SBUF; ScalarE is the
#     attention-phase bottleneck (~1.05us per [128,1024] tile, 64 tiles), so
#     emission keeps two score slots rotating so exp never starves
#   ctxT[d,qi] (+rowsum row via a ones column in v) = [v_h | 1].T @ expT,
#     accumulated over kj one step behind the score/exp stream
#   v projections and next pair's q/k projections are interleaved into the
#     attention loop as PE filler under the ScalarE-bound exp stream
#   normalize per pair: DVE reciprocal of rowsums + K=2 selector matmul
#     broadcast + DVE multiply
#   out_partial[si,dj] = ctxT.T @ WoT  accumulated over the 512 local dims.
#
# PSUM budget (8 banks): 2 score slots ([128,1024] f32 = 2 banks each) + 2
# ctx slots = 8; projections / normalize / out-proj allocations cycle
# through the same two tag pools.
#
# Input DMAs are consolidated (4 x-chunks + 5 weight transfers vs ~40 small
# DMAs before): each dma_start costs ~0.6-1us of fixed issue/HWDGE time.

import sys

sys.path.insert(0, "/opt/trn_rl_repo")

import numpy as np

import concourse.bass as bass
import concourse.mybir as mybir
import concourse.tile as tile
from concourse import bacc
from concourse.bass_utils import run_bass_kernel_spmd

F32 = mybir.dt.float32
BF16 = mybir.dt.bfloat16
FP8 = mybir.dt.float8e4
EXP = mybir.ActivationFunctionType.Exp
DR = mybir.MatmulPerfMode.DoubleRow

P = 128          # SBUF partitions
B = 4            # batch
S = 1024         # sequence length
D = 1024         # model dim
DL = 512         # local head dims per core (8 heads x 64)
HD = 64          # head dim
NHL = 8          # heads per core
KT = D // P      # contraction tiles for projections (8)
SIT = S // P     # si tiles (8)
DJT = DL // P    # dj tiles / head pairs (4)
KJT = S // P     # kj tiles (8)
HALF = 512       # qi half width (PSUM-bank-sized matmul N)
NQH = S // HALF  # 2
SCALE = 1.0 / 8.0  # 1/sqrt(HD)

PROFILE = False          # set True (e.g. from test.py) to capture an NTFF trace
LAST_RESULTS = None      # BassKernelResults of the most recent run


def _build_program(phase="full", reps=1):
    # phase: cumulative prefix for timing attribution:
    #   "io" = input DMAs only, "proj" = +projections, "attn" = +attention,
    #   "norm" = +normalize, "full" = everything
    # reps: unroll the whole body N times inside one program -- marginal
    #   between reps gives device body time without dispatch overhead.
    LV = {"io": 0, "proj": 1, "attn": 2, "norm": 3, "full": 4}[phase]
    nc = bacc.Bacc(
        "TRN2",
        target_bir_lowering=False,
        debug=False,
        enable_asserts=True,
        num_devices=8,
    )

    xT_d = nc.dram_tensor("xT", [D, S], BF16, kind="ExternalInput").ap()
    wq_d = nc.dram_tensor("wqT", [D, DL], BF16, kind="ExternalInput").ap()
    wk_d = nc.dram_tensor("wkT", [D, DL], BF16, kind="ExternalInput").ap()
    wv_d = nc.dram_tensor("wvT", [D, DL], BF16, kind="ExternalInput").ap()
    wo_d = nc.dram_tensor("woT", [DL, D], BF16, kind="ExternalInput").ap()
    out_d = nc.dram_tensor("out_p", [S, D], BF16, kind="ExternalOutput").ap()

    import ml_dtypes

    sel_np = np.zeros((2, P), dtype=ml_dtypes.bfloat16)
    sel_np[0, 0:64] = 1.0
    sel_np[1, 64:128] = 1.0

    with tile.TileContext(nc) as tc:
        with nc.allow_low_precision(reason="bf16 matmul operands"):
            for rep in range(reps):
                _emit_body(
                    nc, tc, rep, LV,
                    xT_d, wq_d, wk_d, wv_d, wo_d, out_d, sel_np,
                )

    nc.compile()
    return nc


def _emit_body(nc, tc, rep, LV, xT_d, wq_d, wk_d, wv_d, wo_d, out_d, sel_np):
    with (
        tc.tile_pool(name=f"pers{rep}", bufs=1) as pers,
        tc.tile_pool(name=f"osb{rep}", bufs=3) as osb,
        tc.tile_pool(name=f"ps{rep}", bufs=2, space="PSUM") as pp,
    ):
        # ---- persistent SBUF tensors -------------------------------
        xT_sb = pers.tile([P, KT, S], BF16, name="xT_sb")
        wq_sb = pers.tile([P, KT, DL], BF16, name="wq_sb")
        wk_sb = pers.tile([P, KT, DL], BF16, name="wk_sb")
        wv_sb = pers.tile([P, KT, DL], BF16, name="wv_sb")
        wo_sb = pers.tile([P, DJT, D], BF16, name="wo_sb")
        qT_sb = pers.tile([P, DJT, S], BF16, name="qT_sb")
        kT_sb = pers.tile([P, DJT, S], BF16, name="kT_sb")
        # v staged with a ones column per head: [si-tile, head, 65]
        # (fp8 would halve the ctx matmul time via DoubleRow but costs
        # ~3e-2 rel err -- over the 2e-2 budget; bf16 measures ~3e-3)
        vst = pers.tile([P, SIT, NHL, HD + 1], BF16, name="vst")
        ctx_sb = pers.tile([P, DJT, S], BF16, name="ctx_sb")
        # all-ones tile: row 64 is the K=1 stationary operand that
        # broadcasts each head's 1/rowsum across 64 partitions
        ones_sb = pers.tile([P, HD], BF16, name="ones_sb")

        nc.vector.memset(ones_sb[:], 1.0)
        nc.vector.memset(vst[:, :, :, HD : HD + 1], 1.0)

        # ---- input DMAs (ordering = transfer priority) -------------
        # dj halves of wq/wk first so pair-0 projections start early,
        # then x in 4 chunks (projection k-loops pipeline against them).
        nc.sync.dma_start(
            wq_sb[:, :, 0:256],
            wq_d[:, 0:256].rearrange("(t p) d -> p t d", p=P),
        )
        nc.sync.dma_start(
            wk_sb[:, :, 0:256],
            wk_d[:, 0:256].rearrange("(t p) d -> p t d", p=P),
        )
        for c in range(4):
            nc.sync.dma_start(
                xT_sb[:, 2 * c : 2 * c + 2, :],
                xT_d[c * 256 : (c + 1) * 256, :].rearrange(
                    "(t p) s -> p t s", p=P
                ),
            )
        nc.sync.dma_start(
            wv_sb[:], wv_d.rearrange("(t p) d -> p t d", p=P)
        )
        nc.sync.dma_start(
            wq_sb[:, :, 256:DL],
            wq_d[:, 256:DL].rearrange("(t p) d -> p t d", p=P),
        )
        nc.sync.dma_start(
            wk_sb[:, :, 256:DL],
            wk_d[:, 256:DL].rearrange("(t p) d -> p t d", p=P),
        )
        nc.sync.dma_start(
            wo_sb[:], wo_d.rearrange("(t p) d -> p t d", p=P)
        )

        # ---- emission helpers --------------------------------------
        CP = mybir.ActivationFunctionType.Copy
        copy_flip = [0]

        def psum_copy(dest_ap, src_ap):
            # PSUM->SBUF drains gate PSUM slot recycling for the PE filler
            # stream; alternate them between DVE and ScalarE (Copy shares
            # the Exp table set, so no table reload) to halve each engine's
            # queue-head latency
            copy_flip[0] ^= 1
            if copy_flip[0]:
                nc.vector.tensor_copy(dest_ap, src_ap)
            else:
                nc.scalar.activation(dest_ap, src_ap, CP)

        def emit_qk_proj_half(w_sb, dest, pr, sh):
            # half-N chunk so a filler holds its PSUM slot only ~1.7us
            # (a full-width chunk starves the exp stream of score slots)
            pst = pp.tile([P, HALF], F32, tag="sc", name="pst")
            for k in range(KT):
                nc.tensor.matmul(
                    pst[:],
                    w_sb[:, k, pr * P : (pr + 1) * P],
                    xT_sb[:, k, sh * HALF : (sh + 1) * HALF],
                    start=(k == 0),
                    stop=(k == KT - 1),
                )
            psum_copy(dest[:, pr, sh * HALF : (sh + 1) * HALF], pst[:])

        def emit_qk_proj_one(w_sb, dest, pr):
            for sh in range(NQH):
                emit_qk_proj_half(w_sb, dest, pr, sh)

        def emit_v(si):
            psv = pp.tile([P, DL], F32, tag="sc", name="psv")
            for k in range(KT):
                nc.tensor.matmul(
                    psv[:],
                    xT_sb[:, k, si * P : (si + 1) * P],
                    wv_sb[:, k, :],
                    start=(k == 0),
                    stop=(k == KT - 1),
                )
            psum_copy(
                vst[:, si, :, 0:HD],
                psv.rearrange("p (h c) -> p h c", c=HD),
            )

        def emit_attention_pair(pr, fillers):
            # fillers: {kj: [callable, ...]} PE work sprinkled into the loop
            heads = (2 * pr, 2 * pr + 1)
            cx = {}
            for h in heads:
                cx[h] = pp.tile([HD + 1, S], F32, tag="cx", name="cx")
            ets = {}

            def scores(kj):
                scs = {}
                for h in heads:
                    scs[h] = pp.tile([P, S], F32, tag="sc", name="sc")
                # interleave even/odd head MMs: disjoint PE row groups
                # (partition bases 0 / 64) run concurrently
                for sh in range(NQH):
                    for h in heads:
                        pb = (h % 2) * 64
                        nc.tensor.matmul(
                            scs[h][:, sh * HALF : (sh + 1) * HALF],
                            kT_sb[pb : pb + 64, pr, kj * P : (kj + 1) * P],
                            qT_sb[pb : pb + 64, pr, sh * HALF : (sh + 1) * HALF],
                            start=True,
                            stop=True,
                        )
                for h in heads:
                    et = osb.tile([P, S], BF16, tag="exp", name="et", bufs=4)
                    nc.scalar.activation(et[:], scs[h][:], EXP, scale=SCALE)
                    ets[(h, kj)] = et

            def ctx(kj):
                for h in heads:
                    et = ets.pop((h, kj))
                    for sh in range(NQH):
                        nc.tensor.matmul(
                            cx[h][:, sh * HALF : (sh + 1) * HALF],
                            vst[:, kj, h, :],
                            et[:, sh * HALF : (sh + 1) * HALF],
                            start=(kj == 0),
                            stop=(kj == KJT - 1),
                        )

            for kj in range(KJT):
                for f in fillers.get(kj, ()):
                    f()
                scores(kj)
                if kj >= 1:
                    ctx(kj - 1)
            ctx(KJT - 1)

            # PSUM can't be a DMA source: bounce via SBUF (DVE), then DMA
            # shifts partitions into the pair layout for the out-projection.
            # The reciprocal runs in place on the rowsum row (partition 64)
            # right on the DVE that produced it -- no DMA round trip.
            csts = {}
            for h in heads:
                pb = (h % 2) * 64
                cst = osb.tile([HD + 1, S], BF16, tag="cst", name="cst", bufs=5)
                psum_copy(cst[:], cx[h][:])
                nc.sync.dma_start(ctx_sb[pb : pb + 64, pr, :], cst[0:HD, :])
                if LV >= 3:
                    nc.vector.reciprocal(cst[HD : HD + 1, :], cst[HD : HD + 1, :])
                csts[h] = cst

            def norm():
                # K=1 matmuls broadcast 1/rowsum (partition 64 of cst) over
                # the head's 64 ctx partitions; emitted a little into the
                # NEXT pair so the PE never waits on the reciprocal.
                rb = pp.tile([P, S], F32, tag="sc", name="rb")
                for sh in range(NQH):
                    for h in heads:
                        pb = (h % 2) * 64
                        nc.tensor.matmul(
                            rb[pb : pb + 64, sh * HALF : (sh + 1) * HALF],
                            ones_sb[64:65, :],
                            csts[h][HD : HD + 1, sh * HALF : (sh + 1) * HALF],
                            start=True,
                            stop=True,
                        )
                nc.vector.tensor_mul(ctx_sb[:, pr, :], ctx_sb[:, pr, :], rb[:])

            return norm if LV >= 3 else (lambda: None)

        # ---- emission schedule -------------------------------------
        if LV >= 1:
            emit_qk_proj_one(wq_sb, qT_sb, 0)
            emit_qk_proj_one(wk_sb, kT_sb, 0)

        def qk_chunks(p):
            return {
                1: [lambda: emit_qk_proj_half(wq_sb, qT_sb, p, 0)],
                3: [lambda: emit_qk_proj_half(wq_sb, qT_sb, p, 1)],
                5: [lambda: emit_qk_proj_half(wk_sb, kT_sb, p, 0)],
                7: [lambda: emit_qk_proj_half(wk_sb, kT_sb, p, 1)],
            }

        if LV >= 2:
            # pair 0 carries the v projections (one si tile per kj, one
            # step ahead of the ctx that consumes it) plus pair 1's q/k;
            # each pair's normalize is deferred into the next pair's loop
            # (kj==2) so the PE never stalls on the reciprocal chain
            fillers0 = {kj: [lambda si=kj: emit_v(si)] for kj in range(KJT)}
            for kj, fs in qk_chunks(1).items():
                fillers0[kj] = fillers0[kj] + fs
            norm0 = emit_attention_pair(0, fillers0)
            f1 = qk_chunks(2)
            f1[2] = f1.get(2, []) + [norm0]
            norm1 = emit_attention_pair(1, f1)
            f2 = qk_chunks(3)
            f2[2] = f2.get(2, []) + [norm1]
            norm2 = emit_attention_pair(2, f2)
            norm3 = emit_attention_pair(3, {2: [norm2]})
            norm3()
        elif LV == 1:
            for pr in range(1, DJT):
                emit_qk_proj_one(wq_sb, qT_sb, pr)
                emit_qk_proj_one(wk_sb, kT_sb, pr)
            for si in range(SIT):
                emit_v(si)

        # ---- out-projection ----------------------------------------
        if LV < 4:
            # dummy output writer so the variant still has an output
            dum = osb.tile([2, HD], BF16, tag="dum", bufs=1)
            nc.vector.tensor_copy(dum[:], ones_sb[0:2, :])
            nc.gpsimd.dma_start(out_d[0:2, 0:HD], dum[:])
        for si in range(SIT if LV >= 4 else 0):
            op = pp.tile([P, S], F32, tag="sc", name="op")
            for pi in range(DJT):
                for dh in range(NQH):
                    nc.tensor.matmul(
                        op[:, dh * HALF : (dh + 1) * HALF],
                        ctx_sb[:, pi, si * P : (si + 1) * P],
                        wo_sb[:, pi, dh * HALF : (dh + 1) * HALF],
                        start=(pi == 0),
                        stop=(pi == DJT - 1),
                    )
            ot = osb.tile([P, S], BF16, tag="ot", name="ot", bufs=2)
            psum_copy(ot[:], op[:])
            nc.sync.dma_start(out_d[si * P : (si + 1) * P, :], ot[:])


_NC_CACHE = {}


def _get_program(phase="full", reps=1):
    key = (phase, reps)
    if key not in _NC_CACHE:
        _NC_CACHE[key] = _build_program(phase, reps)
    return _NC_CACHE[key]


def make_in_maps(x, Wq, Wk, Wv, Wo):
    import ml_dtypes

    bf = ml_dtypes.bfloat16
    x = np.asarray(x, np.float32)
    Wq = np.asarray(Wq, np.float32)
    Wk = np.asarray(Wk, np.float32)
    Wv = np.asarray(Wv, np.float32)
    Wo = np.asarray(Wo, np.float32)
    in_maps = []
    for c in range(8):
        bi, g = divmod(c, 2)
        rs = slice(g * DL, (g + 1) * DL)
        in_maps.append(
            {
                "xT": np.ascontiguousarray(x[bi].T).astype(bf),
                "wqT": np.ascontiguousarray(Wq[rs, :].T).astype(bf),
                "wkT": np.ascontiguousarray(Wk[rs, :].T).astype(bf),
                "wvT": np.ascontiguousarray(Wv[rs, :].T).astype(bf),
                "woT": np.ascontiguousarray(Wo[:, rs].T).astype(bf),
            }
        )
    return in_maps


_EXEC_CACHE = {}


def _get_exec(nc):
    # AOT-compiled sharded executable (bass effect suppressed -> C++ fast
    # dispatch) so repeated kernel() calls don't re-trace/re-compile.
    # Mirrors concourse.bass2jax.run_bass_via_pjrt's multi-core path.
    if id(nc) in _EXEC_CACHE:
        return _EXEC_CACHE[id(nc)]
    import jax
    from jax.sharding import Mesh, NamedSharding, PartitionSpec
    from jax.experimental.shard_map import shard_map

    from concourse.bass2jax import (
        _bass_exec_p,
        fast_dispatch_compile,
        install_neuronx_cc_hook,
        partition_id_tensor,
    )

    install_neuronx_cc_hook()
    n_cores = 8
    partition_name = nc.partition_id_tensor.name if nc.partition_id_tensor else None
    in_names, out_names, out_avals, zero_outs = [], [], [], []
    for alloc in nc.m.functions[0].allocations:
        if not isinstance(alloc, mybir.MemoryLocationSet):
            continue
        name = alloc.memorylocations[0].name
        if alloc.kind == "ExternalInput":
            if name != partition_name:
                in_names.append((name, tuple(alloc.tensor_shape),
                                 mybir.dt.np(alloc.dtype)))
        elif alloc.kind == "ExternalOutput":
            shape = tuple(alloc.tensor_shape)
            dtype = mybir.dt.np(alloc.dtype)
            out_avals.append(jax.core.ShapedArray(shape, dtype))
            zero_outs.append((shape, dtype))
            out_names.append(name)
    n_params = len(in_names)
    n_outs = len(out_avals)
    all_in_names = [n for n, _, _ in in_names] + list(out_names)
    if partition_name is not None:
        all_in_names.append(partition_name)
    donate = tuple(range(n_params, n_params + n_outs))

    def _body(*args):
        operands = list(args)
        if partition_name is not None:
            operands.append(partition_id_tensor())
        return tuple(
            _bass_exec_p.bind(
                *operands,
                out_avals=tuple(out_avals),
                in_names=tuple(all_in_names),
                out_names=tuple(out_names),
                lowering_input_output_aliases=(),
                sim_require_finite=True,
                sim_require_nnan=True,
                nc=nc,
            )
        )

    devices = jax.devices()[:n_cores]
    mesh = Mesh(np.asarray(devices), ("core",))
    sh = NamedSharding(mesh, PartitionSpec("core"))
    in_specs = (PartitionSpec("core"),) * (n_params + n_outs)
    out_specs = (PartitionSpec("core"),) * len(out_names)

    def _gaval(shape, dtype):
        return jax.ShapeDtypeStruct(
            (n_cores * shape[0], *shape[1:]), dtype, sharding=sh
        )

    avals = [_gaval(s, d) for _, s, d in in_names]
    avals += [_gaval(s, d) for s, d in zero_outs]

    def _compile():
        return (
            jax.jit(
                shard_map(
                    _body, mesh=mesh, in_specs=in_specs,
                    out_specs=out_specs, check_rep=False,
                ),
                donate_argnums=donate,
                keep_unused=True,
            )
            .lower(*avals)
            .compile()
        )

    sharded = fast_dispatch_compile(_compile)
    entry = (sharded, [n for n, _, _ in in_names], out_names, zero_outs, sh)
    _EXEC_CACHE[id(nc)] = entry
    return entry


def _run_fast(nc, in_maps):
    import jax

    sharded, in_names, out_names, zero_outs, sh = _get_exec(nc)
    n_cores = len(in_maps)
    concat_in = [
        jax.device_put(
            np.concatenate([np.asarray(m[n]) for m in in_maps], axis=0), sh
        )
        for n in in_names
    ]
    zeros = [
        jax.device_put(
            np.zeros((n_cores * s[0], *s[1:]), d),
            sh,
        )
        for s, d in zero_outs
    ]
    out_arrs = sharded(*concat_in, *zeros)
    return [
        {
            name: np.asarray(out_arrs[i]).reshape(
                n_cores, -1, *np.asarray(out_arrs[i]).shape[1:]
            )[c]
            for i, name in enumerate(out_names)
        }
        for c in range(n_cores)
    ]


def kernel(x, Wq, Wk, Wv, Wo, bo):
    global LAST_RESULTS
    x = np.asarray(x, dtype=np.float32)
    Wq = np.asarray(Wq, dtype=np.float32)
    Wk = np.asarray(Wk, dtype=np.float32)
    Wv = np.asarray(Wv, dtype=np.float32)
    Wo = np.asarray(Wo, dtype=np.float32)
    bo = np.asarray(bo, dtype=np.float32)

    nc = _get_program()
    in_maps = make_in_maps(x, Wq, Wk, Wv, Wo)
    # retry on transient device errors (e.g. NRT_EXEC_UNIT_UNRECOVERABLE
    # from a previous run wedging a core)
    import time as _time

    res = None
    for attempt in range(3):
        try:
            parts8 = _run_fast(nc, in_maps)
            break
        except Exception:  # noqa: BLE001
            parts8 = None
            if attempt == 2:
                break
            _time.sleep(5)
    if parts8 is None:
        for attempt in range(3):
            try:
                res = run_bass_kernel_spmd(
                    nc, in_maps, core_ids=list(range(8)), trace=PROFILE
                )
                break
            except Exception:  # noqa: BLE001
                if attempt == 2:
                    raise
                _time.sleep(20)
        LAST_RESULTS = res
        parts8 = res.results
    parts = [np.asarray(r["out_p"], dtype=np.float32) for r in parts8]
    out = np.empty((B, S, D), dtype=np.float32)
    for bi in range(B):
        out[bi] = parts[2 * bi] + parts[2 * bi + 1] + bo[None, :]
    return out
